# revision 8
# baseline (speedup 1.0000x reference)
"""Trainium2 kernel for nn_JointLikelyhood_Gumbel (NB joint likelihood + Gumbel copula).

Self-contained: kernel(**inputs) takes full inputs, shards across 8 NeuronCores
(data-parallel over the batch), runs one SPMD Bass program, returns the scalar.

Math: per row i and margin j in {1,2}:
  p1   = clip(tanh(p[:,0]), 1e-4, .9999)          (shared across j)
  logp_j = lgamma(y_j+r_j) - lgamma(y_j+1) - lgamma(r_j) + r_j*log1p(-p1) + y_j*log(p1)
  u_j  = clip(sum_{k<=y_j} pmf_j(k), 1e-6, 1-1e-6)
  theta = max(relu(p[:,1])+1, 1.00001)
  ll   = logp_1 + logp_2 - ((-ln u_1)^theta + (-ln u_2)^theta)^(1/theta)
  out  = -mean(ll)

Device strategy: the pmf row is generated with a hardware prefix-scan using the
recurrence pmf(k) = pmf(k-1) * (p + p*(r-1)/k), masked at k>y, then reduced.
The sum is truncated at K = min(y+1, k_cut) where k_cut is the first k past the
mode with logpmf < -104 (terms beyond underflow to exactly 0 in fp32, matching
the fp32 reference). Rows are sorted by max(K1,K2) and packed into 128-row
tiles so each tile's scan width is near its rows' own K. logp at y uses a
shift-8 Stirling series for lgamma. All per-element math runs on-device; the
host only plans the packing (using input values solely to choose provably
fp32-exact truncation points) and averages per-row lls.
"""

import math
from contextlib import ExitStack

import numpy as np

B = 16384
MAX_Y = 4096
NCORE = 8
P = 128
RPC = B // NCORE            # 2048 rows per core
NT = RPC // P               # 16 row-tiles per core
EPS = 1e-6
LGAMMA_CUT = -104.0         # below this, exp() is 0.0 in fp32 (incl. subnormals)
HALF_LN2PI = 0.9189385332046727


# ---------------------------------------------------------------- host planning

def _np_lgamma(z):
    """float64 lgamma, vectorized (scipy-free fallback of scipy.special.gammaln)."""
    z = np.asarray(z, dtype=np.float64)
    prod = np.ones_like(z)
    for i in range(8):
        prod = prod * (z + i)
    w = z + 8.0
    u = 1.0 / w
    u2 = u * u
    s = u * (1.0 / 12.0 - u2 * (1.0 / 360.0 - u2 * (1.0 / 1260.0)))
    return (w - 0.5) * np.log(w) - w + HALF_LN2PI + s - np.log(prod)


def _logpmf64(k, r, p):
    return (_np_lgamma(k + r) - _np_lgamma(k + 1.0) - _np_lgamma(r)
            + r * np.log1p(-p) + k * np.log(p))


def _k_cutoffs(r, p, y):
    """Smallest exclusive end K = min(y+1, first k past mode with logpmf < -104)."""
    mode = np.ceil(np.maximum((r - 1.0) * p / (1.0 - p), 0.0)) + 1.0
    lo = np.minimum(mode, y)
    hi = y
    no_cut = _logpmf64(y, r, p) >= LGAMMA_CUT
    for _ in range(16):
        mid = np.floor((lo + hi) / 2.0)
        below = _logpmf64(mid, r, p) < LGAMMA_CUT
        hi = np.where(below, mid, hi)
        lo = np.where(below, lo, mid + 1.0)
    K = np.where(no_cut, y + 1.0, lo)
    return np.maximum(K, 1.0).astype(np.int64)


def _plan(r, p, target):
    """Sort/pack rows; returns (per-core input dicts, W table, ll weight)."""
    r64 = r.astype(np.float64)
    p64 = p.astype(np.float64)
    y64 = target.astype(np.float64)
    rc = np.maximum(r64, 1e-4)
    p1 = np.clip(np.tanh(p64[:, 0]), 1e-4, 0.9999)

    K1 = _k_cutoffs(rc[:, 0], p1, y64[:, 0])
    K2 = _k_cutoffs(rc[:, 1], p1, y64[:, 1])
    order = np.argsort(np.maximum(K1, K2), kind="stable")

    wtab = np.zeros((NT, 2), np.int64)
    for t in range(NT):
        blk = order[t * NCORE * P:(t + 1) * NCORE * P]
        wtab[t, 0] = min(MAX_Y, max(8, int(math.ceil(K1[blk].max() / 8.0)) * 8))
        wtab[t, 1] = min(MAX_Y, max(8, int(math.ceil(K2[blk].max() / 8.0)) * 8))

    rf = r.astype(np.float32)
    pf = p.astype(np.float32)
    yf = target.astype(np.float32)

    per_core = []
    for c in range(NCORE):
        rows = order[c::NCORE]  # 2048 rows, sorted; tile t = rows[t*128:(t+1)*128]

        def pack2(a1, a2):
            out = np.empty((P, 2 * NT), np.float32)
            for t in range(NT):
                blk = rows[t * P:(t + 1) * P]
                out[:, t] = a1[blk]
                out[:, NT + t] = a2[blk]
            return out

        def pack1(a):
            out = np.empty((P, NT), np.float32)
            for t in range(NT):
                out[:, t] = a[rows[t * P:(t + 1) * P]]
            return out

        per_core.append({
            "rs": pack2(rf[:, 0], rf[:, 1]),
            "ys": pack2(yf[:, 0], yf[:, 1]),
            "p0d": pack2(pf[:, 0], pf[:, 0]),
            "prho": pack1(pf[:, 1]),
        })
    return per_core, wtab


# ---------------------------------------------------------------- device program

def _emit_lgamma(nc, sm, z, tag, shift=8):
    """Shifted-Stirling lgamma on a [P, C] fp32 tile; returns the output tile.

    shift=8 covers z >= 1e-4; shift=4 is enough for z >= ~0.9 and keeps the
    shift product below the scalar engine's Ln range (2^64) for z up to ~4200.
    """
    import concourse.mybir as mybir
    f32 = mybir.dt.float32
    ACT = mybir.ActivationFunctionType
    C = z.shape[1]

    prod = sm.tile([P, C], f32, tag=f"{tag}_prod")
    nc.vector.tensor_copy(prod, z)
    tmp = sm.tile([P, C], f32, tag=f"{tag}_tmp")
    for i in range(1, shift):
        nc.vector.tensor_scalar_add(tmp, z, float(i))
        nc.vector.tensor_mul(prod, prod, tmp)
    lnprod = sm.tile([P, C], f32, tag=f"{tag}_lnprod")
    nc.scalar.activation(lnprod, prod, ACT.Ln)

    w = sm.tile([P, C], f32, tag=f"{tag}_w")
    nc.vector.tensor_scalar_add(w, z, float(shift))
    lnw = sm.tile([P, C], f32, tag=f"{tag}_lnw")
    nc.scalar.activation(lnw, w, ACT.Ln)
    u = sm.tile([P, C], f32, tag=f"{tag}_u")
    nc.vector.reciprocal(u, w)
    u2 = sm.tile([P, C], f32, tag=f"{tag}_u2")
    nc.vector.tensor_mul(u2, u, u)
    s1 = sm.tile([P, C], f32, tag=f"{tag}_s1")
    nc.vector.tensor_scalar(s1, u2, -1.0 / 1260.0, 1.0 / 360.0,
                            mybir.AluOpType.mult, mybir.AluOpType.add)
    nc.vector.tensor_mul(s1, u2, s1)
    nc.vector.tensor_scalar(s1, s1, -1.0, 1.0 / 12.0,
                            mybir.AluOpType.mult, mybir.AluOpType.add)
    nc.vector.tensor_mul(s1, u, s1)               # s1 = series tail
    # main = (w - 0.5)*ln(w) - w + HALF_LN2PI
    nc.vector.tensor_scalar_add(tmp, w, -0.5)
    nc.vector.tensor_mul(tmp, tmp, lnw)
    nc.vector.tensor_sub(tmp, tmp, w)
    out = sm.tile([P, C], f32, tag=f"{tag}_out")
    nc.vector.tensor_scalar_add(out, tmp, HALF_LN2PI)
    nc.vector.tensor_add(out, out, s1)
    nc.vector.tensor_sub(out, out, lnprod)
    return out


def _emit_kernel(nc, tc, ctx, wtab):
    import concourse.bass as bass  # noqa: F401
    import concourse.mybir as mybir
    f32 = mybir.dt.float32
    i32 = mybir.dt.int32
    ACT = mybir.ActivationFunctionType
    OP = mybir.AluOpType
    AX = mybir.AxisListType

    rs_d = nc.dram_tensor("rs", [P, 2 * NT], f32, kind="ExternalInput")
    ys_d = nc.dram_tensor("ys", [P, 2 * NT], f32, kind="ExternalInput")
    p0d_d = nc.dram_tensor("p0d", [P, 2 * NT], f32, kind="ExternalInput")
    prho_d = nc.dram_tensor("prho", [P, NT], f32, kind="ExternalInput")
    ll_d = nc.dram_tensor("ll_out", [P, NT], f32, kind="ExternalOutput")

    wmax = int(wtab.max())
    const = ctx.enter_context(tc.tile_pool(name="const", bufs=1))
    sm = ctx.enter_context(tc.tile_pool(name="sm", bufs=1))
    rpool = ctx.enter_context(tc.tile_pool(name="ratio", bufs=2))
    mpool = ctx.enter_context(tc.tile_pool(name="mask", bufs=2))
    spool = ctx.enter_context(tc.tile_pool(name="scan", bufs=2))

    # ---- constants: iota_f[k]=k, recipk[k]=1/max(k,1), over [P, wmax]
    iota_i = const.tile([P, wmax], i32, tag="iota_i")
    nc.gpsimd.iota(iota_i, pattern=[[1, wmax]], base=0, channel_multiplier=0)
    iota_f = const.tile([P, wmax], f32, tag="iota_f")
    nc.vector.tensor_copy(iota_f, iota_i)
    recipk = const.tile([P, wmax], f32, tag="recipk")
    nc.vector.tensor_scalar_max(recipk, iota_f, 1.0)
    nc.vector.reciprocal(recipk, recipk)

    # ---- load inputs
    rs = const.tile([P, 2 * NT], f32, tag="rs")
    nc.sync.dma_start(out=rs, in_=rs_d.ap())
    ys = const.tile([P, 2 * NT], f32, tag="ys")
    nc.sync.dma_start(out=ys, in_=ys_d.ap())
    p0d = const.tile([P, 2 * NT], f32, tag="p0d")
    nc.sync.dma_start(out=p0d, in_=p0d_d.ap())
    prho = const.tile([P, NT], f32, tag="prho")
    nc.sync.dma_start(out=prho, in_=prho_d.ap())

    # ---- per-row preamble (stacked [P, 2*NT]; col = j*NT + t)
    rcs = sm.tile([P, 2 * NT], f32, tag="rcs")
    nc.vector.tensor_scalar_max(rcs, rs, 1e-4)
    p1d = sm.tile([P, 2 * NT], f32, tag="p1d")
    nc.scalar.activation(p1d, p0d, ACT.Tanh)
    nc.vector.tensor_scalar(p1d, p1d, 1e-4, 0.9999, OP.max, OP.min)
    logp1 = sm.tile([P, 2 * NT], f32, tag="logp1")
    nc.scalar.activation(logp1, p1d, ACT.Ln)
    om = sm.tile([P, 2 * NT], f32, tag="om")
    nc.vector.tensor_scalar(om, p1d, -1.0, 1.0, OP.mult, OP.add)
    logom = sm.tile([P, 2 * NT], f32, tag="logom")
    nc.scalar.activation(logom, om, ACT.Ln)

    pm1 = sm.tile([P, 2 * NT], f32, tag="pm1")        # p*(rc-1)
    nc.vector.tensor_scalar_add(pm1, rcs, -1.0)
    nc.vector.tensor_mul(pm1, pm1, p1d)
    rlo = sm.tile([P, 2 * NT], f32, tag="rlo")        # rc*log(1-p)
    nc.vector.tensor_mul(rlo, rcs, logom)
    pmf0 = sm.tile([P, 2 * NT], f32, tag="pmf0")      # (1-p)^rc
    nc.scalar.activation(pmf0, rlo, ACT.Exp)
    mb = sm.tile([P, 2 * NT], f32, tag="mb")          # sigmoid mask bias
    nc.vector.tensor_scalar(mb, ys, 1e4, 5e3, OP.mult, OP.add)

    theta = sm.tile([P, NT], f32, tag="theta")
    nc.scalar.activation(theta, prho, ACT.Relu)
    nc.vector.tensor_scalar(theta, theta, 1.0, 1.00001, OP.add, OP.max)
    rth = sm.tile([P, NT], f32, tag="rth")
    nc.vector.reciprocal(rth, theta)

    # ---- logp_j at y (Stirling lgammas), stacked
    zyr = sm.tile([P, 2 * NT], f32, tag="zyr")
    nc.vector.tensor_add(zyr, ys, rcs)
    zy1 = sm.tile([P, 2 * NT], f32, tag="zy1")
    nc.vector.tensor_scalar_add(zy1, ys, 1.0)
    lg_yr = _emit_lgamma(nc, sm, zyr, "lgyr", shift=5)
    lg_y1 = _emit_lgamma(nc, sm, zy1, "lgy1", shift=5)
    lg_r = _emit_lgamma(nc, sm, rcs, "lgr", shift=8)

    logp = sm.tile([P, 2 * NT], f32, tag="logp")
    nc.vector.tensor_sub(logp, lg_yr, lg_y1)
    nc.vector.tensor_sub(logp, logp, lg_r)
    nc.vector.tensor_add(logp, logp, rlo)
    ylp = sm.tile([P, 2 * NT], f32, tag="ylp")
    nc.vector.tensor_mul(ylp, ys, logp1)
    nc.vector.tensor_add(logp, logp, ylp)

    # ---- main loop: scan-generated pmf rows, masked, reduced
    u = sm.tile([P, 2 * NT], f32, tag="u")
    for t in range(NT):
        for j in range(2):
            col = j * NT + t
            W = int(wtab[t, j])
            ratio = rpool.tile([P, wmax], f32, tag="ratio")
            nc.scalar.activation(ratio[:, :W], recipk[:, :W], ACT.Identity,
                                 bias=p1d[:, col:col + 1],
                                 scale=pm1[:, col:col + 1])
            nc.vector.memset(ratio[:, 0:1], 1.0)
            mask = mpool.tile([P, wmax], f32, tag="mask")
            nc.scalar.activation(mask[:, :W], iota_f[:, :W], ACT.Sigmoid,
                                 bias=mb[:, col:col + 1], scale=-1e4)
            scano = spool.tile([P, wmax], f32, tag="scan")
            nc.vector.tensor_tensor_scan(scano[:, :W], ratio[:, :W], mask[:, :W],
                                         initial=pmf0[:, col:col + 1],
                                         op0=OP.mult, op1=OP.mult)
            nc.vector.tensor_reduce(u[:, col:col + 1], scano[:, :W],
                                    axis=AX.X, op=OP.add)

    # ---- tail: copula + assembly
    nc.vector.tensor_scalar(u, u, EPS, 1.0 - EPS, OP.max, OP.min)
    lu = sm.tile([P, 2 * NT], f32, tag="lu")
    nc.scalar.activation(lu, u, ACT.Ln)
    llu = sm.tile([P, 2 * NT], f32, tag="llu")
    nc.scalar.activation(llu, lu, ACT.Ln, scale=-1.0)   # ln(-ln u)
    thd = sm.tile([P, 2 * NT], f32, tag="thd")
    nc.vector.tensor_copy(thd[:, :NT], theta)
    nc.vector.tensor_copy(thd[:, NT:], theta)
    nc.vector.tensor_mul(llu, llu, thd)
    tj = sm.tile([P, 2 * NT], f32, tag="tj")
    nc.scalar.activation(tj, llu, ACT.Exp)              # (-ln u)^theta

    s = sm.tile([P, NT], f32, tag="s")
    nc.vector.tensor_add(s, tj[:, :NT], tj[:, NT:])
    nc.vector.tensor_scalar_max(s, s, 1e-38)  # guard Ln(0) if both t_j underflow
    lgs = sm.tile([P, NT], f32, tag="lgs")
    nc.scalar.activation(lgs, s, ACT.Ln)
    nc.vector.tensor_mul(lgs, lgs, rth)
    pw = sm.tile([P, NT], f32, tag="pw")
    nc.scalar.activation(pw, lgs, ACT.Exp)              # (t1+t2)^(1/theta)

    ll = sm.tile([P, NT], f32, tag="ll")
    nc.vector.tensor_add(ll, logp[:, :NT], logp[:, NT:])
    nc.vector.tensor_sub(ll, ll, pw)
    nc.sync.dma_start(out=ll_d.ap(), in_=ll)


def _build(wtab):
    import concourse.bacc as bacc
    import concourse.tile as tile

    # Bacc (not raw Bass): its compile() runs generate_event_semaphores, which
    # splits multi-wait instructions to satisfy the TRN2 1-wait-per-instruction
    # hardware constraint.
    nc = bacc.Bacc("TRN2", target_bir_lowering=False, debug=False)
    with tile.TileContext(nc) as tc:
        with ExitStack() as ctx:
            _emit_kernel(nc, tc, ctx, wtab)
    nc.compile()
    return nc


# ---------------------------------------------------------------- entry point

def kernel(r, p, target):
    from concourse.bass_utils import run_bass_kernel_spmd

    r = np.asarray(r)
    p = np.asarray(p)
    target = np.asarray(target)
    per_core, wtab = _plan(r, p, target)

    nc = _build(wtab)
    res = run_bass_kernel_spmd(nc, per_core, core_ids=list(range(NCORE)))
    total = 0.0
    for c in range(NCORE):
        total += res.results[c]["ll_out"].astype(np.float64).sum()
    return np.float32(-total / B)


# revision 10
# speedup vs baseline: 1.0287x; 1.0287x over previous
"""Trainium2 kernel for nn_JointLikelyhood_Gumbel (NB joint likelihood + Gumbel copula).

Self-contained: kernel(**inputs) takes full inputs, shards across 8 NeuronCores
(data-parallel over the batch), runs one SPMD Bass program, returns the scalar.

Math: per row i and margin j in {1,2}:
  p1   = clip(tanh(p[:,0]), 1e-4, .9999)          (shared across j)
  logp_j = lgamma(y_j+r_j) - lgamma(y_j+1) - lgamma(r_j) + r_j*log1p(-p1) + y_j*log(p1)
  u_j  = clip(sum_{k<=y_j} pmf_j(k), 1e-6, 1-1e-6)
  theta = max(relu(p[:,1])+1, 1.00001)
  ll   = logp_1 + logp_2 - ((-ln u_1)^theta + (-ln u_2)^theta)^(1/theta)
  out  = -mean(ll)

Device strategy: the pmf row is generated with a hardware prefix-scan using the
recurrence pmf(k) = pmf(k-1) * (p + p*(r-1)/k), masked at k>y, then reduced.
The sum is truncated at K = min(y+1, k_cut) where k_cut is the first k past the
mode with logpmf < -104 (terms beyond underflow to exactly 0 in fp32, matching
the fp32 reference). Rows are sorted by max(K1,K2) and packed into 128-row
tiles so each tile's scan width is near its rows' own K. logp at y uses a
shift-8 Stirling series for lgamma. All per-element math runs on-device; the
host only plans the packing (using input values solely to choose provably
fp32-exact truncation points) and averages per-row lls.
"""

import math
from contextlib import ExitStack

import numpy as np

B = 16384
MAX_Y = 4096
NCORE = 8
P = 128
RPC = B // NCORE            # 2048 rows per core
NT = RPC // P               # 16 row-tiles per core
EPS = 1e-6
LGAMMA_CUT = -104.0         # below this, exp() is 0.0 in fp32 (incl. subnormals)
HALF_LN2PI = 0.9189385332046727


# ---------------------------------------------------------------- host planning

def _np_lgamma(z):
    """float64 lgamma, vectorized (scipy-free fallback of scipy.special.gammaln)."""
    z = np.asarray(z, dtype=np.float64)
    prod = np.ones_like(z)
    for i in range(8):
        prod = prod * (z + i)
    w = z + 8.0
    u = 1.0 / w
    u2 = u * u
    s = u * (1.0 / 12.0 - u2 * (1.0 / 360.0 - u2 * (1.0 / 1260.0)))
    return (w - 0.5) * np.log(w) - w + HALF_LN2PI + s - np.log(prod)


def _logpmf64(k, r, p):
    return (_np_lgamma(k + r) - _np_lgamma(k + 1.0) - _np_lgamma(r)
            + r * np.log1p(-p) + k * np.log(p))


def _k_cutoffs(r, p, y):
    """Smallest exclusive end K = min(y+1, first k past mode with logpmf < -104)."""
    mode = np.ceil(np.maximum((r - 1.0) * p / (1.0 - p), 0.0)) + 1.0
    lo = np.minimum(mode, y)
    hi = y
    no_cut = _logpmf64(y, r, p) >= LGAMMA_CUT
    for _ in range(16):
        mid = np.floor((lo + hi) / 2.0)
        below = _logpmf64(mid, r, p) < LGAMMA_CUT
        hi = np.where(below, mid, hi)
        lo = np.where(below, lo, mid + 1.0)
    K = np.where(no_cut, y + 1.0, lo)
    return np.maximum(K, 1.0).astype(np.int64)


def _plan(r, p, target):
    """Sort/pack rows; returns (per-core input dicts, W table, ll weight)."""
    r64 = r.astype(np.float64)
    p64 = p.astype(np.float64)
    y64 = target.astype(np.float64)
    rc = np.maximum(r64, 1e-4)
    p1 = np.clip(np.tanh(p64[:, 0]), 1e-4, 0.9999)

    K1 = _k_cutoffs(rc[:, 0], p1, y64[:, 0])
    K2 = _k_cutoffs(rc[:, 1], p1, y64[:, 1])
    order = np.argsort(np.maximum(K1, K2), kind="stable")

    wtab = np.zeros((NT, 2), np.int64)
    for t in range(NT):
        blk = order[t * NCORE * P:(t + 1) * NCORE * P]
        wtab[t, 0] = min(MAX_Y, max(8, int(math.ceil(K1[blk].max() / 8.0)) * 8))
        wtab[t, 1] = min(MAX_Y, max(8, int(math.ceil(K2[blk].max() / 8.0)) * 8))

    # per-(tile,j): does any row need the y-mask? (y-truncated with padding)
    need_mask = np.zeros((NT, 2), bool)
    Ks = (K1, K2)
    ys64 = (y64[:, 0], y64[:, 1])
    for t in range(NT):
        blk = order[t * NCORE * P:(t + 1) * NCORE * P]
        for j in range(2):
            K = Ks[j][blk]
            yy = ys64[j][blk]
            need_mask[t, j] = bool(np.any((K == yy + 1) & (wtab[t, j] > K)))

    rf = r.astype(np.float32)
    pf = p.astype(np.float32)
    yf = target.astype(np.float32)

    per_core = []
    for c in range(NCORE):
        rows = order[c::NCORE]  # 2048 rows, sorted; tile t = rows[t*128:(t+1)*128]

        def pack2(a1, a2):
            out = np.empty((P, 2 * NT), np.float32)
            for t in range(NT):
                blk = rows[t * P:(t + 1) * P]
                out[:, t] = a1[blk]
                out[:, NT + t] = a2[blk]
            return out

        def pack1(a):
            out = np.empty((P, NT), np.float32)
            for t in range(NT):
                out[:, t] = a[rows[t * P:(t + 1) * P]]
            return out

        per_core.append({
            "rs": pack2(rf[:, 0], rf[:, 1]),
            "ys": pack2(yf[:, 0], yf[:, 1]),
            "p0d": pack2(pf[:, 0], pf[:, 0]),
            "prho": pack1(pf[:, 1]),
        })
    return per_core, wtab, need_mask


# ---------------------------------------------------------------- device program

def _emit_lgamma(nc, sm, z, tag, shift=8):
    """Shifted-Stirling lgamma on a [P, C] fp32 tile; returns the output tile.

    shift=8 covers z >= 1e-4; shift=4 is enough for z >= ~0.9 and keeps the
    shift product below the scalar engine's Ln range (2^64) for z up to ~4200.
    """
    import concourse.mybir as mybir
    f32 = mybir.dt.float32
    ACT = mybir.ActivationFunctionType
    C = z.shape[1]

    prod = sm.tile([P, C], f32, tag=f"{tag}_prod")
    nc.vector.tensor_copy(prod, z)
    tmp = sm.tile([P, C], f32, tag=f"{tag}_tmp")
    for i in range(1, shift):
        nc.vector.tensor_scalar_add(tmp, z, float(i))
        nc.vector.tensor_mul(prod, prod, tmp)
    lnprod = sm.tile([P, C], f32, tag=f"{tag}_lnprod")
    nc.scalar.activation(lnprod, prod, ACT.Ln)

    w = sm.tile([P, C], f32, tag=f"{tag}_w")
    nc.vector.tensor_scalar_add(w, z, float(shift))
    lnw = sm.tile([P, C], f32, tag=f"{tag}_lnw")
    nc.scalar.activation(lnw, w, ACT.Ln)
    u = sm.tile([P, C], f32, tag=f"{tag}_u")
    nc.vector.reciprocal(u, w)
    u2 = sm.tile([P, C], f32, tag=f"{tag}_u2")
    nc.vector.tensor_mul(u2, u, u)
    s1 = sm.tile([P, C], f32, tag=f"{tag}_s1")
    nc.vector.tensor_scalar(s1, u2, -1.0 / 1260.0, 1.0 / 360.0,
                            mybir.AluOpType.mult, mybir.AluOpType.add)
    nc.vector.tensor_mul(s1, u2, s1)
    nc.vector.tensor_scalar(s1, s1, -1.0, 1.0 / 12.0,
                            mybir.AluOpType.mult, mybir.AluOpType.add)
    nc.vector.tensor_mul(s1, u, s1)               # s1 = series tail
    # main = (w - 0.5)*ln(w) - w + HALF_LN2PI
    nc.vector.tensor_scalar_add(tmp, w, -0.5)
    nc.vector.tensor_mul(tmp, tmp, lnw)
    nc.vector.tensor_sub(tmp, tmp, w)
    out = sm.tile([P, C], f32, tag=f"{tag}_out")
    nc.vector.tensor_scalar_add(out, tmp, HALF_LN2PI)
    nc.vector.tensor_add(out, out, s1)
    nc.vector.tensor_sub(out, out, lnprod)
    return out


def _emit_kernel(nc, tc, ctx, wtab, need_mask):
    import concourse.bass as bass  # noqa: F401
    import concourse.mybir as mybir
    f32 = mybir.dt.float32
    i32 = mybir.dt.int32
    ACT = mybir.ActivationFunctionType
    OP = mybir.AluOpType
    AX = mybir.AxisListType

    rs_d = nc.dram_tensor("rs", [P, 2 * NT], f32, kind="ExternalInput")
    ys_d = nc.dram_tensor("ys", [P, 2 * NT], f32, kind="ExternalInput")
    p0d_d = nc.dram_tensor("p0d", [P, 2 * NT], f32, kind="ExternalInput")
    prho_d = nc.dram_tensor("prho", [P, NT], f32, kind="ExternalInput")
    ll_d = nc.dram_tensor("ll_out", [P, NT], f32, kind="ExternalOutput")

    wmax = int(wtab.max())
    const = ctx.enter_context(tc.tile_pool(name="const", bufs=1))
    sm = ctx.enter_context(tc.tile_pool(name="sm", bufs=1))
    rpool = ctx.enter_context(tc.tile_pool(name="ratio", bufs=2))
    mpool = ctx.enter_context(tc.tile_pool(name="mask", bufs=2))
    spool = ctx.enter_context(tc.tile_pool(name="scan", bufs=2))

    # ---- constants: iota_f[k]=k, recipk[k]=1/max(k,1), over [P, wmax]
    iota_i = const.tile([P, wmax], i32, tag="iota_i")
    nc.gpsimd.iota(iota_i, pattern=[[1, wmax]], base=0, channel_multiplier=0)
    iota_f = const.tile([P, wmax], f32, tag="iota_f")
    nc.vector.tensor_copy(iota_f, iota_i)
    recipk = const.tile([P, wmax], f32, tag="recipk")
    nc.vector.tensor_scalar_max(recipk, iota_f, 1.0)
    nc.vector.reciprocal(recipk, recipk)

    # ---- load inputs
    rs = const.tile([P, 2 * NT], f32, tag="rs")
    nc.sync.dma_start(out=rs, in_=rs_d.ap())
    ys = const.tile([P, 2 * NT], f32, tag="ys")
    nc.sync.dma_start(out=ys, in_=ys_d.ap())
    p0d = const.tile([P, 2 * NT], f32, tag="p0d")
    nc.sync.dma_start(out=p0d, in_=p0d_d.ap())
    prho = const.tile([P, NT], f32, tag="prho")
    nc.sync.dma_start(out=prho, in_=prho_d.ap())

    # ---- per-row preamble (stacked [P, 2*NT]; col = j*NT + t)
    rcs = sm.tile([P, 2 * NT], f32, tag="rcs")
    nc.vector.tensor_scalar_max(rcs, rs, 1e-4)
    p1d = sm.tile([P, 2 * NT], f32, tag="p1d")
    nc.scalar.activation(p1d, p0d, ACT.Tanh)
    nc.vector.tensor_scalar(p1d, p1d, 1e-4, 0.9999, OP.max, OP.min)
    logp1 = sm.tile([P, 2 * NT], f32, tag="logp1")
    nc.scalar.activation(logp1, p1d, ACT.Ln)
    om = sm.tile([P, 2 * NT], f32, tag="om")
    nc.vector.tensor_scalar(om, p1d, -1.0, 1.0, OP.mult, OP.add)
    logom = sm.tile([P, 2 * NT], f32, tag="logom")
    nc.scalar.activation(logom, om, ACT.Ln)

    pm1 = sm.tile([P, 2 * NT], f32, tag="pm1")        # p*(rc-1)
    nc.vector.tensor_scalar_add(pm1, rcs, -1.0)
    nc.vector.tensor_mul(pm1, pm1, p1d)
    rlo = sm.tile([P, 2 * NT], f32, tag="rlo")        # rc*log(1-p)
    nc.vector.tensor_mul(rlo, rcs, logom)
    pmf0 = sm.tile([P, 2 * NT], f32, tag="pmf0")      # (1-p)^rc
    nc.scalar.activation(pmf0, rlo, ACT.Exp)
    mb = sm.tile([P, 2 * NT], f32, tag="mb")          # sigmoid mask bias
    nc.vector.tensor_scalar(mb, ys, 1e4, 5e3, OP.mult, OP.add)
    # scan initial state seeded so out[:,0] = ratio0*init = pmf0 (ratio0 = p*rc)
    init2 = sm.tile([P, 2 * NT], f32, tag="init2")
    nc.vector.tensor_mul(init2, p1d, rcs)
    nc.vector.reciprocal(init2, init2)
    nc.vector.tensor_mul(init2, init2, pmf0)
    ones = const.tile([P, wmax], f32, tag="ones")     # data1 for unmasked scans
    nc.vector.memset(ones, 1.0)

    theta = sm.tile([P, NT], f32, tag="theta")
    nc.scalar.activation(theta, prho, ACT.Relu)
    nc.vector.tensor_scalar(theta, theta, 1.0, 1.00001, OP.add, OP.max)
    rth = sm.tile([P, NT], f32, tag="rth")
    nc.vector.reciprocal(rth, theta)

    # ---- logp_j at y (Stirling lgammas), stacked
    zyr = sm.tile([P, 2 * NT], f32, tag="zyr")
    nc.vector.tensor_add(zyr, ys, rcs)
    zy1 = sm.tile([P, 2 * NT], f32, tag="zy1")
    nc.vector.tensor_scalar_add(zy1, ys, 1.0)
    lg_yr = _emit_lgamma(nc, sm, zyr, "lgyr", shift=5)
    lg_y1 = _emit_lgamma(nc, sm, zy1, "lgy1", shift=5)
    lg_r = _emit_lgamma(nc, sm, rcs, "lgr", shift=8)

    logp = sm.tile([P, 2 * NT], f32, tag="logp")
    nc.vector.tensor_sub(logp, lg_yr, lg_y1)
    nc.vector.tensor_sub(logp, logp, lg_r)
    nc.vector.tensor_add(logp, logp, rlo)
    ylp = sm.tile([P, 2 * NT], f32, tag="ylp")
    nc.vector.tensor_mul(ylp, ys, logp1)
    nc.vector.tensor_add(logp, logp, ylp)

    # ---- main loop: scan-generated pmf rows, masked, reduced
    u = sm.tile([P, 2 * NT], f32, tag="u")
    for t in range(NT):
        for j in range(2):
            col = j * NT + t
            W = int(wtab[t, j])
            ratio = rpool.tile([P, wmax], f32, tag="ratio")
            nc.scalar.activation(ratio[:, :W], recipk[:, :W], ACT.Identity,
                                 bias=p1d[:, col:col + 1],
                                 scale=pm1[:, col:col + 1])
            if need_mask[t, j]:
                mask = mpool.tile([P, wmax], f32, tag="mask")
                nc.scalar.activation(mask[:, :W], iota_f[:, :W], ACT.Sigmoid,
                                     bias=mb[:, col:col + 1], scale=-1e4)
                data1 = mask
            else:
                data1 = ones
            scano = spool.tile([P, wmax], f32, tag="scan")
            nc.vector.tensor_tensor_scan(scano[:, :W], ratio[:, :W], data1[:, :W],
                                         initial=init2[:, col:col + 1],
                                         op0=OP.mult, op1=OP.mult)
            nc.vector.tensor_reduce(u[:, col:col + 1], scano[:, :W],
                                    axis=AX.X, op=OP.add)

    # ---- tail: copula + assembly
    nc.vector.tensor_scalar(u, u, EPS, 1.0 - EPS, OP.max, OP.min)
    lu = sm.tile([P, 2 * NT], f32, tag="lu")
    nc.scalar.activation(lu, u, ACT.Ln)
    llu = sm.tile([P, 2 * NT], f32, tag="llu")
    nc.scalar.activation(llu, lu, ACT.Ln, scale=-1.0)   # ln(-ln u)
    thd = sm.tile([P, 2 * NT], f32, tag="thd")
    nc.vector.tensor_copy(thd[:, :NT], theta)
    nc.vector.tensor_copy(thd[:, NT:], theta)
    nc.vector.tensor_mul(llu, llu, thd)
    tj = sm.tile([P, 2 * NT], f32, tag="tj")
    nc.scalar.activation(tj, llu, ACT.Exp)              # (-ln u)^theta

    s = sm.tile([P, NT], f32, tag="s")
    nc.vector.tensor_add(s, tj[:, :NT], tj[:, NT:])
    nc.vector.tensor_scalar_max(s, s, 1e-38)  # guard Ln(0) if both t_j underflow
    lgs = sm.tile([P, NT], f32, tag="lgs")
    nc.scalar.activation(lgs, s, ACT.Ln)
    nc.vector.tensor_mul(lgs, lgs, rth)
    pw = sm.tile([P, NT], f32, tag="pw")
    nc.scalar.activation(pw, lgs, ACT.Exp)              # (t1+t2)^(1/theta)

    ll = sm.tile([P, NT], f32, tag="ll")
    nc.vector.tensor_add(ll, logp[:, :NT], logp[:, NT:])
    nc.vector.tensor_sub(ll, ll, pw)
    nc.sync.dma_start(out=ll_d.ap(), in_=ll)


def _build(wtab, need_mask):
    import concourse.bacc as bacc
    import concourse.tile as tile

    # Bacc (not raw Bass): its compile() runs generate_event_semaphores, which
    # splits multi-wait instructions to satisfy the TRN2 1-wait-per-instruction
    # hardware constraint.
    nc = bacc.Bacc("TRN2", target_bir_lowering=False, debug=False)
    with tile.TileContext(nc) as tc:
        with ExitStack() as ctx:
            _emit_kernel(nc, tc, ctx, wtab, need_mask)
    nc.compile()
    return nc


# ---------------------------------------------------------------- entry point

def kernel(r, p, target):
    from concourse.bass_utils import run_bass_kernel_spmd

    r = np.asarray(r)
    p = np.asarray(p)
    target = np.asarray(target)
    per_core, wtab, need_mask = _plan(r, p, target)

    nc = _build(wtab, need_mask)
    res = run_bass_kernel_spmd(nc, per_core, core_ids=list(range(NCORE)))
    total = 0.0
    for c in range(NCORE):
        total += res.results[c]["ll_out"].astype(np.float64).sum()
    return np.float32(-total / B)
